# revision 7
# baseline (speedup 1.0000x reference)
"""Trainium2 Bass kernel for nn_Attention_56470230008033.

Multi-head self-attention (B=2, N=2048, C=1024, H=16 heads, D=64),
k = v = q, full qkv projection + output projection.

Sharding over 8 NeuronCores: data parallel on batch (2) x tensor
parallel on heads (4 head-groups of 4 heads). Each core computes, for
its (batch b, head group g):
  - qkv = x @ Wqkv[:, cols(g)]         (bf16 matmul, fp32 accum)
  - per head: logits^T = K^T.T @ Q^T, P^T = exp(logits^T * 1/8)
    (no max-subtraction: logits are bounded ~|8| for this problem)
  - o_u^T / sums via [V | 1] ones-column trick, normalize
  - y_partial = o_hat @ Wproj[rows(g), :]
Host sums the 4 partials per batch and adds b_proj.
"""

import sys

for _p in ("/opt/trn_rl_repo", "/opt/pypackages"):
    if _p not in sys.path:
        sys.path.append(_p)

import numpy as np

B, N, C, H = 2, 2048, 1024, 16
D = C // H            # 64 head dim
NCORES = 8
HPC = 4               # heads per core
F = HPC * D           # 256 features per core
NT = N // 128         # 16 token tiles
CT = C // 128         # 8 contraction tiles
NCH = N // 512        # 4 free-dim chunks of 512

_CACHE = {}


def _build():
    from concourse import bacc, bass, mybir, tile, masks

    F32 = mybir.dt.float32
    BF16 = mybir.dt.bfloat16
    AF = mybir.ActivationFunctionType

    nc = bacc.Bacc(
        "TRN2",
        target_bir_lowering=False,
        debug=False,
        enable_asserts=False,
        num_devices=NCORES,
    )
    x_d = nc.dram_tensor("x", [N, C], F32, kind="ExternalInput")
    wqk_d = nc.dram_tensor("wqk", [C, 2 * F], F32, kind="ExternalInput")
    wv_d = nc.dram_tensor("wv", [C, F], F32, kind="ExternalInput")
    wp_d = nc.dram_tensor("wp", [F, C], F32, kind="ExternalInput")
    bqk_d = nc.dram_tensor("bqk", [2 * F, 1], F32, kind="ExternalInput")
    bv_d = nc.dram_tensor("bv", [1, F], F32, kind="ExternalInput")
    y_d = nc.dram_tensor("y", [N, C], F32, kind="ExternalOutput")

    with tile.TileContext(nc) as tc:
        from contextlib import ExitStack

        with ExitStack() as ctx:
            const = ctx.enter_context(tc.tile_pool(name="const", bufs=1))
            persist = ctx.enter_context(tc.tile_pool(name="persist", bufs=1))

            ident = const.tile([128, 128], BF16, name="ident", tag="ident")
            masks.make_identity(nc, ident[:])

            # persistent SBUF tensors (bf16 compute copies)
            # xTall: x^T, laid out as 8 c-tiles of [128, 2048] side by side
            xTall = persist.tile([128, CT * N], BF16, name="xTall", tag="xTall")
            wqk = [persist.tile([128, 2 * F], BF16, name=f"wqk{c}", tag=f"wqk{c}") for c in range(CT)]
            wv = [persist.tile([128, F], BF16, name=f"wv{c}", tag=f"wv{c}") for c in range(CT)]
            wp = [persist.tile([D, C], BF16, name=f"wp{h}", tag=f"wp{h}") for h in range(HPC)]
            # qkT[0..1] = Q^T tiles (256 rows), qkT[2..3] = K^T tiles
            qkT = [persist.tile([128, N], BF16, name=f"qkT{f}", tag=f"qkT{f}") for f in range(4)]
            # V with interleaved ones column per head: cols [65h .. 65h+64]
            vaug = [persist.tile([128, 65 * HPC], BF16, name=f"vaug{t}", tag=f"vaug{t}") for t in range(NT)]
            oT = [persist.tile([D, N], BF16, name=f"oT{h}", tag=f"oT{h}") for h in range(HPC)]
            bqk_sb = [const.tile([128, 1], F32, name=f"bqk{f}", tag=f"bqk{f}") for f in range(4)]
            bvb = const.tile([128, F], F32, name="bvb", tag="bvb")

            # x^T view: [128, c-tile, n]
            xT = xTall.rearrange("p (c n) -> p c n", c=CT)

            # ---- phase A: x load (2 n-tiles per DMA), cast, transpose ----
            # x rows (tp*256 + i*128 + p) -> xs[p, i*1024 + c]
            x_view = x_d.ap().rearrange("(tp i p) c -> tp p i c", tp=NT // 2, i=2)
            with tc.tile_pool(name="tpsum", bufs=2, space=bass.MemorySpace.PSUM) as tpsum, \
                 tc.tile_pool(name="vpsum", bufs=1, space=bass.MemorySpace.PSUM) as vpsum, \
                 tc.tile_pool(name="qkpsum", bufs=1, space=bass.MemorySpace.PSUM) as qkpsum, \
                 tc.tile_pool(name="xload", bufs=2) as xload, \
                 tc.tile_pool(name="xbp", bufs=2) as xbp, \
                 tc.tile_pool(name="wstage", bufs=2) as wstage:
                for tp in range(NT // 2):
                    xs = xload.tile([128, 2048], F32, name="xs", tag="xs")
                    xsv = xs.rearrange("p (i c) -> p i c", i=2)
                    dmae = nc.sync if tp % 2 == 0 else nc.scalar
                    if tp < 2:
                        # finer first transfers so the PE can start sooner
                        for i in range(2):
                            dmae.dma_start(xsv[:, i], x_view[tp][:, i])
                    else:
                        dmae.dma_start(xsv, x_view[tp])
                    for i in range(2):
                        t = 2 * tp + i
                        xb = xbp.tile([128, 1024], BF16, name="xb", tag="xb")
                        nc.vector.tensor_copy(xb[:], xs[:, i * 1024:(i + 1) * 1024])
                        for cq in range(2):  # quad of 4 c-tiles
                            tq = tpsum.tile([128, 512], BF16, name="tq", tag="tq")
                            for j in range(4):
                                c = 4 * cq + j
                                nc.tensor.transpose(
                                    tq[:, j * 128:(j + 1) * 128],
                                    xb[:, c * 128:(c + 1) * 128],
                                    ident[:])
                            # scatter the quad into xT[c][:, t*128:(t+1)*128]
                            nc.scalar.copy(
                                xT[:, 4 * cq:4 * cq + 4, t * 128:(t + 1) * 128],
                                tq.rearrange("p (c n) -> p c n", c=4))
                    # weight loads on the (otherwise idle) gpsimd DMA ring
                    if tp < 4:
                        for c in (2 * tp, 2 * tp + 1):
                            s = wstage.tile([128, 2 * F], F32, name="wqks", tag="wqks")
                            nc.gpsimd.dma_start(s[:], wqk_d.ap()[c * 128:(c + 1) * 128, :])
                            nc.vector.tensor_copy(wqk[c][:], s[:])
                            s2 = wstage.tile([128, F], F32, name="wvs", tag="wvs")
                            nc.gpsimd.dma_start(s2[:], wv_d.ap()[c * 128:(c + 1) * 128, :])
                            nc.vector.tensor_copy(wv[c][:], s2[:])
                    elif tp == 4:
                        for f in range(4):
                            nc.gpsimd.dma_start(bqk_sb[f][:], bqk_d.ap()[f * 128:(f + 1) * 128, :])
                        for h in range(HPC):
                            s3 = wstage.tile([D, C], F32, name="wps", tag="wps")
                            nc.gpsimd.dma_start(s3[:], wp_d.ap()[h * D:(h + 1) * D, :])
                            nc.vector.tensor_copy(wp[h][:], s3[:])
                    elif tp == 5:
                        bv1 = const.tile([1, F], F32, name="bv1", tag="bv1")
                        nc.gpsimd.dma_start(bv1[:], bv_d.ap()[:])
                        nc.gpsimd.partition_broadcast(bvb[:], bv1[:])
                        for t2 in range(NT):
                            for h in range(HPC):
                                nc.vector.memset(vaug[t2][:, 65 * h + 64:65 * h + 65], 1.0)

                # Q^T / K^T projections, K for heads 0-1 first so phase 2 can
                # begin; Q/K for heads 2-3 overlap the start of phase 2 (their
                # PSUM->SBUF copies run on the vector engine since the scalar
                # engine paces phase 2 with exps).
                def qk_proj(f, copy_eng):
                    for nch in range(NCH):
                        qp = qkpsum.tile([128, 512], F32, name="qp", tag="qp")
                        for c in range(CT):
                            nc.tensor.matmul(
                                qp[:],
                                wqk[c][:, f * 128:(f + 1) * 128],
                                xT[:, c, nch * 512:(nch + 1) * 512],
                                start=(c == 0), stop=(c == CT - 1))
                        if copy_eng == "scalar":
                            nc.scalar.activation(
                                qkT[f][:, nch * 512:(nch + 1) * 512], qp[:],
                                AF.Identity, bias=bqk_sb[f][:])
                        else:
                            nc.vector.tensor_scalar_add(
                                qkT[f][:, nch * 512:(nch + 1) * 512], qp[:],
                                bqk_sb[f][:])

                qk_proj(2, "scalar")   # K^T heads 0-1
                qk_proj(0, "scalar")   # Q^T heads 0-1

                # V projection (token-major) per n-tile
                for t in range(NT):
                    vp = vpsum.tile([128, F], F32, name="vp", tag="vp")
                    for c in range(CT):
                        nc.tensor.matmul(
                            vp[:], xT[:, c, t * 128:(t + 1) * 128], wv[c][:],
                            start=(c == 0), stop=(c == CT - 1))
                    for h in range(HPC):
                        nc.vector.tensor_add(
                            vaug[t][:, 65 * h:65 * h + D],
                            vp[:, h * D:(h + 1) * D],
                            bvb[:, h * D:(h + 1) * D])

                qk_proj(3, "vector")   # K^T heads 2-3
                qk_proj(1, "vector")   # Q^T heads 2-3

            # ---- phase 2: per-head attention, split in two n-halves ----
            with tc.tile_pool(name="bpsum", bufs=2, space=bass.MemorySpace.PSUM) as bpsum, \
                 tc.tile_pool(name="cpsum", bufs=2, space=bass.MemorySpace.PSUM) as cpsum, \
                 tc.tile_pool(name="ptp", bufs=6) as ptp, \
                 tc.tile_pool(name="snorm", bufs=2) as snorm:
                for h in range(HPC):
                    qt = qkT[h // 2]
                    kt = qkT[2 + h // 2]
                    rb = D * (h % 2)  # row base within the f-tile
                    for half in range(2):
                        nb = half * 1024
                        cp = cpsum.tile([65, 1024], F32, name="cp", tag="cp")
                        for mt in range(NT):
                            bp = bpsum.tile([128, 1024], F32, name="bp", tag="bp")
                            for sub in range(2):
                                nc.tensor.matmul(
                                    bp[:, sub * 512:(sub + 1) * 512],
                                    kt[rb:rb + D, mt * 128:(mt + 1) * 128],
                                    qt[rb:rb + D, nb + sub * 512:nb + (sub + 1) * 512],
                                    start=True, stop=True)
                            pt = ptp.tile([128, 1024], BF16, name="pt", tag="pt")
                            nc.scalar.activation(pt[:], bp[:], AF.Exp, scale=float(D) ** -0.5)
                            for sub in range(2):
                                nc.tensor.matmul(
                                    cp[:, sub * 512:(sub + 1) * 512],
                                    vaug[mt][:, 65 * h:65 * h + 65],
                                    pt[:, sub * 512:(sub + 1) * 512],
                                    start=(mt == 0), stop=(mt == NT - 1))
                        sr = snorm.tile([1, 1024], F32, name="sr", tag="sr")
                        nc.vector.reciprocal(sr[:], cp[64:65, :])
                        sb = snorm.tile([D, 1024], F32, name="sb", tag="sb")
                        nc.gpsimd.partition_broadcast(sb[:], sr[:])
                        nc.vector.tensor_mul(oT[h][:, nb:nb + 1024], cp[0:D, :], sb[:])

            # ---- phase 3: output projection (partial, head-group rows) ----
            with tc.tile_pool(name="ypsum", bufs=2, space=bass.MemorySpace.PSUM) as ypsum, \
                 tc.tile_pool(name="ysb", bufs=3) as ysb:
                for t in range(NT):
                    yp = ypsum.tile([128, 1024], F32, name="yp", tag="yp")
                    for h in range(HPC):
                        for ch in range(2):
                            nc.tensor.matmul(
                                yp[:, ch * 512:(ch + 1) * 512],
                                oT[h][:, t * 128:(t + 1) * 128],
                                wp[h][:, ch * 512:(ch + 1) * 512],
                                start=(h == 0), stop=(h == HPC - 1))
                    ys = ysb.tile([128, 1024], F32, name="ys", tag="ys")
                    nc.vector.tensor_copy(ys[:], yp[:])
                    nc.sync.dma_start(y_d.ap()[t * 128:(t + 1) * 128, :], ys[:])

    nc.compile()
    return nc


def _get_nc():
    if "nc" not in _CACHE:
        _CACHE["nc"] = _build()
    return _CACHE["nc"]


def _in_maps(q, W_qkv, b_qkv, W_proj):
    maps = []
    for core in range(NCORES):
        b, g = divmod(core, HPC)
        cols = slice(g * F, (g + 1) * F)
        maps.append({
            "x": q[b],
            "wqk": np.ascontiguousarray(
                np.concatenate([W_qkv[:, cols], W_qkv[:, C:][:, cols]], axis=1)),
            "wv": np.ascontiguousarray(W_qkv[:, 2 * C:][:, cols]),
            "wp": np.ascontiguousarray(W_proj[cols, :]),
            "bqk": np.ascontiguousarray(
                np.concatenate([b_qkv[cols], b_qkv[C:][cols]]).reshape(2 * F, 1)),
            "bv": np.ascontiguousarray(b_qkv[2 * C:][cols].reshape(1, F)),
        })
    return maps


def kernel(q, W_qkv, b_qkv, W_proj, b_proj):
    from concourse.bass_utils import run_bass_kernel_spmd

    q = np.ascontiguousarray(np.asarray(q, dtype=np.float32))
    W_qkv = np.ascontiguousarray(np.asarray(W_qkv, dtype=np.float32))
    b_qkv = np.ascontiguousarray(np.asarray(b_qkv, dtype=np.float32))
    W_proj = np.ascontiguousarray(np.asarray(W_proj, dtype=np.float32))
    b_proj = np.ascontiguousarray(np.asarray(b_proj, dtype=np.float32))

    nc = _get_nc()
    res = run_bass_kernel_spmd(nc, _in_maps(q, W_qkv, b_qkv, W_proj),
                               core_ids=list(range(NCORES)))

    out = np.zeros((B, N, C), dtype=np.float32)
    for core in range(NCORES):
        out[core // HPC] += res.results[core]["y"]
    out += b_proj
    return out


# revision 8
# speedup vs baseline: 1.4024x; 1.4024x over previous
"""Trainium2 Bass kernel for nn_Attention_56470230008033.

Multi-head self-attention (B=2, N=2048, C=1024, H=16 heads, D=64),
k = v = q, full qkv projection + output projection.

Sharding over 8 NeuronCores: data parallel on batch (2) x tensor
parallel on heads (4 head-groups of 4 heads). Each core computes, for
its (batch b, head group g):
  - qkv = x @ Wqkv[:, cols(g)]         (bf16 matmul, fp32 accum)
  - per head: logits^T = K^T.T @ Q^T, P^T = exp(logits^T * 1/8)
    (no max-subtraction: logits are bounded ~|8| for this problem)
  - o_u^T / sums via [V | 1] ones-column trick, normalize
  - y_partial = o_hat @ Wproj[rows(g), :]
Host sums the 4 partials per batch and adds b_proj.
"""

import sys

for _p in ("/opt/trn_rl_repo", "/opt/pypackages"):
    if _p not in sys.path:
        sys.path.append(_p)

import numpy as np

B, N, C, H = 2, 2048, 1024, 16
D = C // H            # 64 head dim
NCORES = 8
HPC = 4               # heads per core
F = HPC * D           # 256 features per core
NT = N // 128         # 16 token tiles
CT = C // 128         # 8 contraction tiles
NCH = N // 512        # 4 free-dim chunks of 512

_CACHE = {}


def _build():
    from concourse import bacc, bass, mybir, tile, masks

    F32 = mybir.dt.float32
    BF16 = mybir.dt.bfloat16
    AF = mybir.ActivationFunctionType

    nc = bacc.Bacc(
        "TRN2",
        target_bir_lowering=False,
        debug=False,
        enable_asserts=False,
        num_devices=NCORES,
    )
    x_d = nc.dram_tensor("x", [N, C], F32, kind="ExternalInput")
    wqk_d = nc.dram_tensor("wqk", [C, 2 * F], F32, kind="ExternalInput")
    wv_d = nc.dram_tensor("wv", [C, F], F32, kind="ExternalInput")
    wp_d = nc.dram_tensor("wp", [F, C], F32, kind="ExternalInput")
    bqk_d = nc.dram_tensor("bqk", [2 * F, 1], F32, kind="ExternalInput")
    bv_d = nc.dram_tensor("bv", [1, F], F32, kind="ExternalInput")
    y_d = nc.dram_tensor("y", [N, C], F32, kind="ExternalOutput")

    with tile.TileContext(nc) as tc:
        from contextlib import ExitStack

        with ExitStack() as ctx:
            const = ctx.enter_context(tc.tile_pool(name="const", bufs=1))
            persist = ctx.enter_context(tc.tile_pool(name="persist", bufs=1))

            ident = const.tile([128, 128], BF16, name="ident", tag="ident")
            masks.make_identity(nc, ident[:])

            # persistent SBUF tensors (bf16 compute copies)
            # xTall: x^T, laid out as 8 c-tiles of [128, 2048] side by side
            xTall = persist.tile([128, CT * N], BF16, name="xTall", tag="xTall")
            wqk = [persist.tile([128, 2 * F], BF16, name=f"wqk{c}", tag=f"wqk{c}") for c in range(CT)]
            wv = [persist.tile([128, F], BF16, name=f"wv{c}", tag=f"wv{c}") for c in range(CT)]
            wp = [persist.tile([D, C], BF16, name=f"wp{h}", tag=f"wp{h}") for h in range(HPC)]
            # qkT[0..1] = Q^T tiles (256 rows), qkT[2..3] = K^T tiles
            qkT = [persist.tile([128, N], BF16, name=f"qkT{f}", tag=f"qkT{f}") for f in range(4)]
            # V with interleaved ones column per head: cols [65h .. 65h+64]
            vaug = [persist.tile([128, 65 * HPC], BF16, name=f"vaug{t}", tag=f"vaug{t}") for t in range(NT)]
            oT = [persist.tile([D, N], BF16, name=f"oT{h}", tag=f"oT{h}") for h in range(HPC)]
            bqk_sb = [const.tile([128, 1], F32, name=f"bqk{f}", tag=f"bqk{f}") for f in range(4)]
            bvb = const.tile([128, F], F32, name="bvb", tag="bvb")

            # x^T view: [128, c-tile, n]
            xT = xTall.rearrange("p (c n) -> p c n", c=CT)

            # ---- phase A: x load (2 n-tiles per DMA), cast, transpose ----
            # x rows (tp*256 + i*128 + p) -> xs[p, i*1024 + c]
            x_view = x_d.ap().rearrange("(tp i p) c -> tp p i c", tp=NT // 2, i=2)
            with tc.tile_pool(name="tpsum", bufs=3, space=bass.MemorySpace.PSUM) as tpsum, \
                 tc.tile_pool(name="vpsum", bufs=2, space=bass.MemorySpace.PSUM) as vpsum, \
                 tc.tile_pool(name="qkpsum", bufs=3, space=bass.MemorySpace.PSUM) as qkpsum, \
                 tc.tile_pool(name="xload", bufs=2) as xload, \
                 tc.tile_pool(name="xbp", bufs=2) as xbp, \
                 tc.tile_pool(name="wstage", bufs=2) as wstage:
                for tp in range(NT // 2):
                    xs = xload.tile([128, 2048], F32, name="xs", tag="xs")
                    xsv = xs.rearrange("p (i c) -> p i c", i=2)
                    dmae = nc.sync if tp % 2 == 0 else nc.scalar
                    if tp < 2:
                        # finer first transfers so the PE can start sooner
                        for i in range(2):
                            dmae.dma_start(xsv[:, i], x_view[tp][:, i])
                    else:
                        dmae.dma_start(xsv, x_view[tp])
                    for i in range(2):
                        t = 2 * tp + i
                        xb = xbp.tile([128, 1024], BF16, name="xb", tag="xb")
                        nc.vector.tensor_copy(xb[:], xs[:, i * 1024:(i + 1) * 1024])
                        for cq in range(2):  # quad of 4 c-tiles
                            tq = tpsum.tile([128, 512], BF16, name="tq", tag="tq")
                            for j in range(4):
                                c = 4 * cq + j
                                nc.tensor.transpose(
                                    tq[:, j * 128:(j + 1) * 128],
                                    xb[:, c * 128:(c + 1) * 128],
                                    ident[:])
                            # scatter the quad into xT[c][:, t*128:(t+1)*128]
                            nc.scalar.copy(
                                xT[:, 4 * cq:4 * cq + 4, t * 128:(t + 1) * 128],
                                tq.rearrange("p (c n) -> p c n", c=4))
                    # weight loads on the (otherwise idle) gpsimd DMA ring
                    if tp < 4:
                        for c in (2 * tp, 2 * tp + 1):
                            s = wstage.tile([128, 2 * F], F32, name="wqks", tag="wqks")
                            nc.scalar.dma_start(s[:], wqk_d.ap()[c * 128:(c + 1) * 128, :])
                            nc.vector.tensor_copy(wqk[c][:], s[:])
                            s2 = wstage.tile([128, F], F32, name="wvs", tag="wvs")
                            nc.sync.dma_start(s2[:], wv_d.ap()[c * 128:(c + 1) * 128, :])
                            nc.vector.tensor_copy(wv[c][:], s2[:])
                    elif tp == 4:
                        for f in range(4):
                            nc.sync.dma_start(bqk_sb[f][:], bqk_d.ap()[f * 128:(f + 1) * 128, :])
                        for h in range(HPC):
                            s3 = wstage.tile([D, C], F32, name="wps", tag="wps")
                            nc.scalar.dma_start(s3[:], wp_d.ap()[h * D:(h + 1) * D, :])
                            nc.vector.tensor_copy(wp[h][:], s3[:])
                    elif tp == 5:
                        bv1 = const.tile([1, F], F32, name="bv1", tag="bv1")
                        nc.sync.dma_start(bv1[:], bv_d.ap()[:])
                        nc.gpsimd.partition_broadcast(bvb[:], bv1[:])
                        for t2 in range(NT):
                            for h in range(HPC):
                                nc.vector.memset(vaug[t2][:, 65 * h + 64:65 * h + 65], 1.0)

                # Q^T / K^T projections, K for heads 0-1 first so phase 2 can
                # begin; Q/K for heads 2-3 overlap the start of phase 2 (their
                # PSUM->SBUF copies run on the vector engine since the scalar
                # engine paces phase 2 with exps).
                def qk_proj(f, copy_eng):
                    for nch in range(NCH):
                        qp = qkpsum.tile([128, 512], F32, name="qp", tag="qp")
                        for c in range(CT):
                            nc.tensor.matmul(
                                qp[:],
                                wqk[c][:, f * 128:(f + 1) * 128],
                                xT[:, c, nch * 512:(nch + 1) * 512],
                                start=(c == 0), stop=(c == CT - 1))
                        if copy_eng == "scalar":
                            nc.scalar.activation(
                                qkT[f][:, nch * 512:(nch + 1) * 512], qp[:],
                                AF.Identity, bias=bqk_sb[f][:])
                        else:
                            nc.vector.tensor_scalar_add(
                                qkT[f][:, nch * 512:(nch + 1) * 512], qp[:],
                                bqk_sb[f][:])

                qk_proj(2, "scalar")   # K^T heads 0-1
                qk_proj(0, "scalar")   # Q^T heads 0-1

                # V projection (token-major) per n-tile
                for t in range(NT):
                    vp = vpsum.tile([128, F], F32, name="vp", tag="vp")
                    for c in range(CT):
                        nc.tensor.matmul(
                            vp[:], xT[:, c, t * 128:(t + 1) * 128], wv[c][:],
                            start=(c == 0), stop=(c == CT - 1))
                    for h in range(HPC):
                        nc.vector.tensor_add(
                            vaug[t][:, 65 * h:65 * h + D],
                            vp[:, h * D:(h + 1) * D],
                            bvb[:, h * D:(h + 1) * D])

                qk_proj(3, "scalar")   # K^T heads 2-3
                qk_proj(1, "scalar")   # Q^T heads 2-3

            # ---- phase 2: per-head attention, split in two n-halves ----
            with tc.tile_pool(name="bpsum", bufs=2, space=bass.MemorySpace.PSUM) as bpsum, \
                 tc.tile_pool(name="cpsum", bufs=2, space=bass.MemorySpace.PSUM) as cpsum, \
                 tc.tile_pool(name="ptp", bufs=6) as ptp, \
                 tc.tile_pool(name="snorm", bufs=2) as snorm:
                for h in range(HPC):
                    qt = qkT[h // 2]
                    kt = qkT[2 + h // 2]
                    rb = D * (h % 2)  # row base within the f-tile
                    for half in range(2):
                        nb = half * 1024
                        cp = cpsum.tile([65, 1024], F32, name="cp", tag="cp")
                        for mt in range(NT):
                            bp = bpsum.tile([128, 1024], F32, name="bp", tag="bp")
                            for sub in range(2):
                                nc.tensor.matmul(
                                    bp[:, sub * 512:(sub + 1) * 512],
                                    kt[rb:rb + D, mt * 128:(mt + 1) * 128],
                                    qt[rb:rb + D, nb + sub * 512:nb + (sub + 1) * 512],
                                    start=True, stop=True)
                            pt = ptp.tile([128, 1024], BF16, name="pt", tag="pt")
                            nc.scalar.activation(pt[:], bp[:], AF.Exp, scale=float(D) ** -0.5)
                            for sub in range(2):
                                nc.tensor.matmul(
                                    cp[:, sub * 512:(sub + 1) * 512],
                                    vaug[mt][:, 65 * h:65 * h + 65],
                                    pt[:, sub * 512:(sub + 1) * 512],
                                    start=(mt == 0), stop=(mt == NT - 1))
                        s0 = snorm.tile([1, 1024], F32, name="s0", tag="s0")
                        nc.vector.tensor_copy(s0[:], cp[64:65, :])
                        sr = snorm.tile([1, 1024], F32, name="sr", tag="sr")
                        nc.vector.reciprocal_approx_fast(sr[:], s0[:])
                        sb = snorm.tile([D, 1024], F32, name="sb", tag="sb")
                        nc.gpsimd.partition_broadcast(sb[:], sr[:])
                        nc.vector.tensor_mul(oT[h][:, nb:nb + 1024], cp[0:D, :], sb[:])

            # ---- phase 3: output projection (partial, head-group rows) ----
            with tc.tile_pool(name="ypsum", bufs=2, space=bass.MemorySpace.PSUM) as ypsum, \
                 tc.tile_pool(name="ysb", bufs=3) as ysb:
                for t in range(NT):
                    yp = ypsum.tile([128, 1024], F32, name="yp", tag="yp")
                    for h in range(HPC):
                        for ch in range(2):
                            nc.tensor.matmul(
                                yp[:, ch * 512:(ch + 1) * 512],
                                oT[h][:, t * 128:(t + 1) * 128],
                                wp[h][:, ch * 512:(ch + 1) * 512],
                                start=(h == 0), stop=(h == HPC - 1))
                    ys = ysb.tile([128, 1024], F32, name="ys", tag="ys")
                    nc.vector.tensor_copy(ys[:], yp[:])
                    nc.sync.dma_start(y_d.ap()[t * 128:(t + 1) * 128, :], ys[:])

    nc.compile()
    return nc


def _get_nc():
    if "nc" not in _CACHE:
        _CACHE["nc"] = _build()
    return _CACHE["nc"]


def _in_maps(q, W_qkv, b_qkv, W_proj):
    maps = []
    for core in range(NCORES):
        b, g = divmod(core, HPC)
        cols = slice(g * F, (g + 1) * F)
        maps.append({
            "x": q[b],
            "wqk": np.ascontiguousarray(
                np.concatenate([W_qkv[:, cols], W_qkv[:, C:][:, cols]], axis=1)),
            "wv": np.ascontiguousarray(W_qkv[:, 2 * C:][:, cols]),
            "wp": np.ascontiguousarray(W_proj[cols, :]),
            "bqk": np.ascontiguousarray(
                np.concatenate([b_qkv[cols], b_qkv[C:][cols]]).reshape(2 * F, 1)),
            "bv": np.ascontiguousarray(b_qkv[2 * C:][cols].reshape(1, F)),
        })
    return maps


def kernel(q, W_qkv, b_qkv, W_proj, b_proj):
    from concourse.bass_utils import run_bass_kernel_spmd

    q = np.ascontiguousarray(np.asarray(q, dtype=np.float32))
    W_qkv = np.ascontiguousarray(np.asarray(W_qkv, dtype=np.float32))
    b_qkv = np.ascontiguousarray(np.asarray(b_qkv, dtype=np.float32))
    W_proj = np.ascontiguousarray(np.asarray(W_proj, dtype=np.float32))
    b_proj = np.ascontiguousarray(np.asarray(b_proj, dtype=np.float32))

    nc = _get_nc()
    res = run_bass_kernel_spmd(nc, _in_maps(q, W_qkv, b_qkv, W_proj),
                               core_ids=list(range(NCORES)))

    out = np.zeros((B, N, C), dtype=np.float32)
    for core in range(NCORES):
        out[core // HPC] += res.results[core]["y"]
    out += b_proj
    return out
